# revision 9
# baseline (speedup 1.0000x reference)
"""Distributed multi-head attention (RoPE, non-causal) on 8 TRN2 NeuronCores.

Sharding: tensor-parallel over heads. Core c owns heads {2c, 2c+1}:
  - wq/wk/wv rows c*256:(c+1)*256 (output dim), x replicated (pre-shuffled),
  - attention computed locally per (batch, head),
  - per-(batch, quarter) AllGather of attention outputs (transposed, bf16),
  - each core computes output columns c*256:(c+1)*256 with its wo rows.

v2 structure (vs v1):
  - attn@v computed v-STATIONARY: lhsT = v block [k,hd], rhs = exp [k,q],
    so output lands pre-transposed [hd, q] with N=512 streams; no PE
    transposes and no LDWEIGHTS-bound N=129 matmuls.
  - softmax denominator: pairwise-tree sum of exp tiles over the k-block
    axis on gpsimd+vector, partition-reduced and broadcast in one
    all-ones matmul ([128,128] ones -> psd[q-denominator in every
    partition]), reciprocal + multiply on vector.
  - ScalarE exp throughput (16 x 686ns per unit) exceeds the unit's own
    matmul time, so next-batch projection chunks and prev-batch wo
    quarters are interleaved INTO each attention unit between the scores
    and attn@v groups; attn@v then never waits on the exp stream.
  - quarter-batch AllGather/wo granularity shrinks the serial tail.
  - host pre-shuffles x/weights to [128, ...] layouts so every DMA line
    is >= 8KB contiguous per partition (fast startup).
  - RoPE multiplies read the projection PSUM directly (fp32), the
    add/sub pairs run in bf16 SBUF (DVE 4x mode).
"""

import numpy as np
import ml_dtypes

B, S, D, H = 4, 2048, 2048, 16
HD = 128            # head dim
NCORES = 8
HPC = H // NCORES   # heads per core = 2
OSL = HPC * HD      # per-core o-slice = 256
ROWS = B * S        # 8192 flattened rows
DCH = D // 128      # 16 contraction chunks
SCH = 512           # seq chunk for projections
NBLK = ROWS // SCH  # 16 x column blocks
KB = S // 128       # 16 k-blocks per batch
QC = 512            # q chunk in attention
NQC = S // QC       # 4 quarters per batch
NU = B * NQC        # 16 collective units (quarter batches)

BF16 = ml_dtypes.bfloat16
_NC_CACHE = None


def _build():
    import concourse.bass as bass  # noqa: F401
    import concourse.mybir as mybir
    import concourse.tile as tile
    from concourse import bacc

    fp32 = mybir.dt.float32
    bf16 = mybir.dt.bfloat16

    nc = bacc.Bacc(
        "TRN2",
        target_bir_lowering=False,
        debug=False,
        num_devices=NCORES,
    )

    # Pre-shuffled layouts (host): one contiguous >=8KB run per partition.
    xS = nc.declare_dram_parameter("xS", [128, NBLK, DCH, SCH], bf16,
                                   isOutput=False)
    wqS = nc.declare_dram_parameter("wqS", [128, DCH, OSL], bf16,
                                    isOutput=False)
    wkS = nc.declare_dram_parameter("wkS", [128, DCH, OSL], bf16,
                                    isOutput=False)
    wvS = nc.declare_dram_parameter("wvS", [128, DCH, OSL], bf16,
                                    isOutput=False)
    woS = nc.declare_dram_parameter("woS", [128, DCH, OSL], bf16,
                                    isOutput=False)
    cosd = nc.declare_dram_parameter("cosd", [128, S], bf16, isOutput=False)
    sind = nc.declare_dram_parameter("sind", [128, S], bf16, isOutput=False)
    outp = nc.declare_dram_parameter("out", [OSL, ROWS], fp32, isOutput=True)

    inv_sqrt_hd = 1.0 / float(np.sqrt(HD))

    with tile.TileContext(nc) as tc:
        with (
            tc.tile_pool(name="glob", bufs=1) as glob,
            tc.tile_pool(name="dram", bufs=1, space="DRAM") as dram,
            tc.tile_pool(name="qkv", bufs=2) as qkv,
            tc.tile_pool(name="xtp", bufs=2) as xtp,
            tc.tile_pool(name="attp", bufs=2) as attp,
            tc.tile_pool(name="treep", bufs=2) as treep,
            tc.tile_pool(name="ropep", bufs=2) as ropep,
            tc.tile_pool(name="rcpp", bufs=2) as rcpp,
            tc.tile_pool(name="atp", bufs=2) as atp,
            tc.tile_pool(name="gtp", bufs=2) as gtp,
            tc.tile_pool(name="otp", bufs=2) as otp,
            tc.tile_pool(name="psP", bufs=5, space="PSUM") as psP,
            tc.tile_pool(name="poP", bufs=2, space="PSUM") as poP,
            tc.tile_pool(name="psV", bufs=1, space="PSUM") as psV,
        ):
            ones128 = glob.tile([128, 128], bf16, name="ones128")
            nc.vector.memset(ones128[:], 1.0)

            wq_sb = glob.tile([128, DCH, OSL], bf16, name="wq_sb")
            wk_sb = glob.tile([128, DCH, OSL], bf16, name="wk_sb")
            wv_sb = glob.tile([128, DCH, OSL], bf16, name="wv_sb")
            wo_sb = glob.tile([128, DCH, OSL], bf16, name="wo_sb")
            cosb = glob.tile([128, S], bf16, name="cosb")
            sinb = glob.tile([128, S], bf16, name="sinb")
            # startup order: what the first projection needs, first
            nc.gpsimd.dma_start(wq_sb[:], wqS[:, :, :])
            xt00 = xtp.tile([128, DCH, SCH], bf16, name="xt00", tag="xt")
            nc.gpsimd.dma_start(xt00[:], xS[:, 0, :, :])
            nc.gpsimd.dma_start(cosb[:], cosd[:, :])
            nc.gpsimd.dma_start(sinb[:], sind[:, :])
            nc.gpsimd.dma_start(wk_sb[:], wkS[:, :, :])
            nc.gpsimd.dma_start(wv_sb[:], wvS[:, :, :])
            nc.gpsimd.dma_start(wo_sb[:], woS[:, :, :])

            bounce = [dram.tile([OSL, QC], bf16, name=f"bounce{u}")
                      for u in range(NU)]
            gath = [dram.tile([NCORES * OSL, QC], bf16, addr_space="Shared",
                              name=f"gath{u}") for u in range(NU)]

            def fetch_x(b, sc):
                xt = xtp.tile([128, DCH, SCH], bf16, name="xt", tag="xt")
                nc.gpsimd.dma_start(xt[:], xS[:, b * (S // SCH) + sc, :, :])
                return xt

            proj_result = {}

            def proj_chunks(b, xt_first):
                """Yield 24 chunk-closures projecting batch b (q/k/v)."""
                qt = qkv.tile([128, HPC, S], bf16, name="qt", tag="qt")
                kt = qkv.tile([128, HPC, S], bf16, name="kt", tag="kt")
                vt = qkv.tile([128, KB, HPC, HD], bf16, name="vt", tag="vt")
                proj_result[b] = (qt, kt, vt)
                state = {"xt": xt_first, "xt_next": None}

                def qk_chunk(sc, w_sb, dstT, h):
                    xt = state["xt"]
                    sl = slice(sc * SCH, (sc + 1) * SCH)
                    cosr = cosb[:, sl]
                    sinr = sinb[:, sl]
                    ps = psP.tile([128, SCH], fp32, name="ps", tag="ps")
                    for c in range(DCH):
                        nc.tensor.matmul(
                            ps[:], w_sb[:, c, h * HD:(h + 1) * HD],
                            xt[:, c, :], start=(c == 0), stop=(c == DCH - 1))
                    m1 = ropep.tile([128, SCH], bf16, name="m1", tag="m1")
                    m2 = ropep.tile([128, SCH], bf16, name="m2", tag="m2")
                    # m1 = [tr*cos ; ti*cos]; m2 swapped-halves =
                    # [ti*sin ; tr*sin] so the DVE add/sub below uses equal
                    # SBUF base partitions (the PSUM operand may differ).
                    nc.vector.tensor_mul(m1[:], ps[:], cosr)
                    nc.vector.tensor_mul(
                        m2[0:64, :], ps[64:128, :], sinr[0:64, :])
                    nc.vector.tensor_mul(
                        m2[64:128, :], ps[0:64, :], sinr[64:128, :])
                    nc.vector.tensor_sub(
                        dstT[0:64, h, sl], m1[0:64, :], m2[0:64, :])
                    nc.vector.tensor_add(
                        dstT[64:128, h, sl], m2[64:128, :], m1[64:128, :])

                def v_chunk(sc, pair):
                    xt = state["xt"]
                    psv = psV.tile([128, 2, OSL], fp32, name="psv")
                    for j in range(2):
                        ssb = pair * 2 + j
                        for c in range(DCH):
                            nc.tensor.matmul(
                                psv[:, j, :],
                                xt[:, c, ssb * 128:(ssb + 1) * 128],
                                wv_sb[:, c, :],
                                start=(c == 0), stop=(c == DCH - 1))
                    kb0 = sc * 4 + pair * 2
                    nc.vector.tensor_copy(
                        vt[:, kb0:kb0 + 2, :, :],
                        psv[:].rearrange("p s (h d) -> p s h d", h=HPC))

                def make(fn, *args):
                    return lambda: fn(*args)

                def first_chunk(sc):
                    # rotate in the prefetched x tile and prefetch the next
                    if sc > 0:
                        state["xt"] = state["xt_next"]
                    if sc + 1 < S // SCH:
                        state["xt_next"] = fetch_x(b, sc + 1)
                    qk_chunk(sc, wq_sb, qt, 0)

                for sc in range(S // SCH):
                    yield make(first_chunk, sc)
                    yield make(qk_chunk, sc, wq_sb, qt, 1)
                    yield make(qk_chunk, sc, wk_sb, kt, 0)
                    yield make(qk_chunk, sc, wk_sb, kt, 1)
                    yield make(v_chunk, sc, 0)
                    yield make(v_chunk, sc, 1)

            def allgather(u):
                nc.gpsimd.collective_compute(
                    "AllGather",
                    mybir.AluOpType.bypass,
                    ins=[bounce[u].opt()],
                    outs=[gath[u].opt()],
                    replica_groups=[list(range(NCORES))],
                )

            def wo_quarter(b, qc):
                u = b * NQC + qc
                gh = []
                for dh in range(2):
                    g = gtp.tile([128, DCH // 2, QC], bf16, name="gt",
                                 tag="gt")
                    nc.sync.dma_start(
                        g[:],
                        gath[u][dh * 1024:(dh + 1) * 1024, :]
                        .rearrange("(c p) n -> p c n", p=128))
                    gh.append(g)
                for oc in range(OSL // 128):
                    psw = psP.tile([128, QC], fp32, name="psw", tag="ps")
                    for c in range(DCH):
                        nc.tensor.matmul(
                            psw[:],
                            wo_sb[:, c, oc * 128:(oc + 1) * 128],
                            gh[c // 8][:, c % 8, :],
                            start=(c == 0), stop=(c == DCH - 1))
                    out_t = otp.tile([128, QC], fp32, name="out_t")
                    nc.vector.tensor_copy(out_t[:], psw[:])
                    col0 = b * S + qc * QC
                    nc.sync.dma_start(
                        outp[oc * 128:(oc + 1) * 128, col0:col0 + QC],
                        out_t[:])

            # --- attention with software pipelining -------------------
            # flush state: pending normalize of the previous unit
            pend = []

            def flush_one():
                if not pend:
                    return
                b, h, qc, po, dsum = pend.pop(0)
                psd = psP.tile([128, QC], fp32, name="psd", tag="ps")
                nc.tensor.matmul(psd[:], ones128[:], dsum[:],
                                 start=True, stop=True)
                rcp = rcpp.tile([128, QC], fp32, name="rcp")
                nc.vector.reciprocal(rcp[:], psd[:])
                a_t = atp.tile([128, QC], bf16, name="a_t")
                nc.vector.tensor_mul(a_t[:], po[:], rcp[:])
                u = b * NQC + qc
                nc.gpsimd.dma_start(
                    bounce[u][h * HD:(h + 1) * HD, :], a_t[:])
                if h == 1:
                    allgather(u)

            def attn_unit(b, qt, kt, vt, h, qc, fillers):
                expT = attp.tile([128, KB, QC], bf16, name="expT")
                for kb in range(KB):
                    pss = psP.tile([128, QC], fp32, name="pss", tag="ps")
                    nc.tensor.matmul(
                        pss[:],
                        kt[:, h, kb * 128:(kb + 1) * 128],
                        qt[:, h, qc * QC:(qc + 1) * QC],
                        start=True, stop=True)
                    nc.scalar.activation(
                        expT[:, kb, :], pss[:],
                        mybir.ActivationFunctionType.Exp,
                        scale=inv_sqrt_hd)
                # normalize/ship the previous unit while exps stream
                flush_one()
                # interleave independent matmul work so the attn@v group
                # below never waits on the ScalarE exp chain
                for f in fillers:
                    f()
                po = poP.tile([128, QC], fp32, name="po", tag="po")
                for kb in range(KB):
                    nc.tensor.matmul(
                        po[:], vt[:, kb, h, 0:HD], expT[:, kb, :],
                        start=(kb == 0), stop=(kb == KB - 1))
                # denominator: tree-sum the 16 k-blocks (gpsimd+vector)
                s1 = treep.tile([128, 4, QC], bf16, name="s1", tag="s1")
                s2 = treep.tile([128, 4, QC], bf16, name="s2", tag="s2")
                s4 = treep.tile([128, 2, QC], bf16, name="s4", tag="s4")
                dsum = treep.tile([128, QC], bf16, name="dsum", tag="ds")
                nc.gpsimd.tensor_add(s1[:], expT[:, 0:4, :], expT[:, 4:8, :])
                nc.vector.tensor_add(s2[:], expT[:, 8:12, :],
                                     expT[:, 12:16, :])
                nc.vector.tensor_add(s1[:], s1[:], s2[:])
                nc.gpsimd.tensor_add(s4[:], s1[:, 0:2, :], s1[:, 2:4, :])
                nc.vector.tensor_add(dsum[:], s4[:, 0, :], s4[:, 1, :])
                pend.append((b, h, qc, po, dsum))

            # ---------------- main schedule ---------------------------
            # proj(0) runs unfilled upfront; proj(b+1) and wo(b-1)
            # quarters fill the attention(b) units.
            gens = {}
            gens[0] = proj_chunks(0, xt00)
            for f in gens[0]:
                f()  # issue all 24 chunks of batch 0 now

            for b in range(B):
                if b + 1 < B:
                    xt_n = fetch_x(b + 1, 0)
                    gens[b + 1] = proj_chunks(b + 1, xt_n)
                qt, kt, vt = proj_result[b]
                # wo quarters available to fill batch b's units
                wo_sched = {}
                if b >= 1:
                    # wo of the previous batch: quarters 0..3 at units
                    # 0,2,4,6 (qc=3's AllGather is issued during unit 0)
                    for i, qq in enumerate(range(NQC)):
                        wo_sched.setdefault(2 * i, []).append((b - 1, qq))
                if b == B - 1:
                    # also drain this batch's own early quarters
                    for i, qq in enumerate(range(NQC - 1)):
                        wo_sched.setdefault(2 * qq + 3, []).append((b, qq))
                units = [(h, qc) for qc in range(NQC) for h in range(HPC)]
                for ui, (h, qc) in enumerate(units):
                    fillers = []
                    if b + 1 < B:
                        g = gens[b + 1]
                        for _ in range(3):
                            try:
                                fillers.append(next(g))
                            except StopIteration:
                                break
                    for (wb, wq) in wo_sched.get(ui, []):
                        fillers.append(
                            lambda wb=wb, wq=wq: wo_quarter(wb, wq))
                    attn_unit(b, qt, kt, vt, h, qc, fillers)
                if b + 1 < B:
                    # drain any leftover proj chunks of b+1
                    for f in gens[b + 1]:
                        f()
            flush_one()
            wo_quarter(B - 1, NQC - 1)

    nc.compile()
    return nc


def _shard_inputs(x, freqs_cos, freqs_sin, wq, wk, wv, wo):
    xf = np.asarray(x, dtype=np.float32).reshape(ROWS, D)
    xT = np.ascontiguousarray(xf.T).astype(BF16)  # [D, ROWS]
    # pre-shuffle: xS[p, blk, c, j] = xT[c*128+p, blk*512+j]
    xS = np.ascontiguousarray(
        xT.reshape(DCH, 128, NBLK, SCH).transpose(1, 2, 0, 3))
    fcT = np.asarray(freqs_cos, dtype=np.float32).T  # [64, S]
    fsT = np.asarray(freqs_sin, dtype=np.float32).T
    cosd = np.ascontiguousarray(np.concatenate([fcT, fcT], 0)).astype(BF16)
    sind = np.ascontiguousarray(np.concatenate([fsT, fsT], 0)).astype(BF16)
    # even indices (real half) then odd (imag half), per head
    perm = np.concatenate([np.arange(0, HD, 2), np.arange(1, HD, 2)])

    def shuf(wrows):  # [OSL, D] -> [128, DCH, OSL]
        wT = np.ascontiguousarray(np.asarray(wrows, dtype=np.float32).T)
        return np.ascontiguousarray(
            wT.reshape(DCH, 128, OSL).transpose(1, 0, 2)).astype(BF16)

    in_maps = []
    for c in range(NCORES):
        rows = slice(c * OSL, (c + 1) * OSL)
        wq_c = np.asarray(wq)[rows].reshape(HPC, HD, D)[:, perm, :]
        wk_c = np.asarray(wk)[rows].reshape(HPC, HD, D)[:, perm, :]
        in_maps.append({
            "xS": xS,
            "wqS": shuf(wq_c.reshape(OSL, D)),
            "wkS": shuf(wk_c.reshape(OSL, D)),
            "wvS": shuf(np.asarray(wv)[rows]),
            "woS": shuf(np.asarray(wo)[rows]),
            "cosd": cosd,
            "sind": sind,
        })
    return in_maps


def run(inputs, trace=False, trace_cores=None):
    """Build (cached), run on 8 cores; returns (full_output, results)."""
    global _NC_CACHE
    from concourse.bass_utils import run_bass_kernel_spmd
    if _NC_CACHE is None:
        _NC_CACHE = _build()
    in_maps = _shard_inputs(**inputs)
    res = run_bass_kernel_spmd(
        _NC_CACHE, in_maps, core_ids=list(range(NCORES)), trace=trace,
        trace_cores=trace_cores)
    parts = [np.ascontiguousarray(
        np.asarray(res.results[c]["out"], dtype=np.float32).T)
        for c in range(NCORES)]
    full = np.concatenate(parts, axis=1).reshape(B, S, D)
    return full, res


def kernel(x, freqs_cos, freqs_sin, wq, wk, wv, wo):
    full, _ = run(dict(x=x, freqs_cos=freqs_cos, freqs_sin=freqs_sin,
                       wq=wq, wk=wk, wv=wv, wo=wo))
    return full


# revision 16
# speedup vs baseline: 1.0675x; 1.0675x over previous
"""Distributed multi-head attention (RoPE, non-causal) on 8 TRN2 NeuronCores.

Sharding: tensor-parallel over heads. Core c owns heads {2c, 2c+1}:
  - wq/wk/wv rows c*256:(c+1)*256 (output dim), x replicated (pre-shuffled),
  - attention computed locally per (batch, head),
  - per-(batch, quarter) AllGather of attention outputs (transposed, bf16),
  - each core computes output columns c*256:(c+1)*256 with its wo rows.

v2 structure (vs v1):
  - attn@v computed v-STATIONARY: lhsT = v block [k,hd], rhs = exp [k,q],
    so output lands pre-transposed [hd, q] with N=512 streams; no PE
    transposes and no LDWEIGHTS-bound N=129 matmuls.
  - softmax denominator: pairwise-tree sum of exp tiles over the k-block
    axis on gpsimd+vector, partition-reduced and broadcast in one
    all-ones matmul ([128,128] ones -> psd[q-denominator in every
    partition]), reciprocal + multiply on vector.
  - ScalarE exp throughput (16 x 686ns per unit) exceeds the unit's own
    matmul time, so next-batch projection chunks and prev-batch wo
    quarters are interleaved INTO each attention unit between the scores
    and attn@v groups; attn@v then never waits on the exp stream.
  - quarter-batch AllGather/wo granularity shrinks the serial tail.
  - host pre-shuffles x/weights to [128, ...] layouts so every DMA line
    is >= 8KB contiguous per partition (fast startup).
  - RoPE multiplies read the projection PSUM directly (fp32), the
    add/sub pairs run in bf16 SBUF (DVE 4x mode).
"""

import numpy as np
import ml_dtypes

B, S, D, H = 4, 2048, 2048, 16
HD = 128            # head dim
NCORES = 8
HPC = H // NCORES   # heads per core = 2
OSL = HPC * HD      # per-core o-slice = 256
ROWS = B * S        # 8192 flattened rows
DCH = D // 128      # 16 contraction chunks
SCH = 512           # seq chunk for projections
NBLK = ROWS // SCH  # 16 x column blocks
KB = S // 128       # 16 k-blocks per batch
QC = 512            # q chunk in attention
NQC = S // QC       # 4 quarters per batch
HB = S // 2         # half-batch column span of one AllGather unit
NU = B * 2          # 8 collective units (half batches)

BF16 = ml_dtypes.bfloat16
_NC_CACHE = None


def _build():
    import concourse.bass as bass  # noqa: F401
    import concourse.mybir as mybir
    import concourse.tile as tile
    from concourse import bacc

    fp32 = mybir.dt.float32
    bf16 = mybir.dt.bfloat16

    nc = bacc.Bacc(
        "TRN2",
        target_bir_lowering=False,
        debug=False,
        num_devices=NCORES,
    )

    # Pre-shuffled layouts (host): one contiguous >=8KB run per partition.
    xS = nc.declare_dram_parameter("xS", [128, NBLK, DCH, SCH], bf16,
                                   isOutput=False)
    wqS = nc.declare_dram_parameter("wqS", [128, DCH, OSL], bf16,
                                    isOutput=False)
    wkS = nc.declare_dram_parameter("wkS", [128, DCH, OSL], bf16,
                                    isOutput=False)
    wvS = nc.declare_dram_parameter("wvS", [128, DCH, OSL], bf16,
                                    isOutput=False)
    woS = nc.declare_dram_parameter("woS", [128, DCH, OSL], bf16,
                                    isOutput=False)
    cosd = nc.declare_dram_parameter("cosd", [128, S], bf16, isOutput=False)
    sind = nc.declare_dram_parameter("sind", [128, S], bf16, isOutput=False)
    outp = nc.declare_dram_parameter("out", [OSL, ROWS], fp32, isOutput=True)

    inv_sqrt_hd = 1.0 / float(np.sqrt(HD))

    with tile.TileContext(nc) as tc:
        with (
            tc.tile_pool(name="glob", bufs=1) as glob,
            tc.tile_pool(name="dram", bufs=1, space="DRAM") as dram,
            tc.tile_pool(name="qkv", bufs=2) as qkv,
            tc.tile_pool(name="xtp", bufs=2) as xtp,
            tc.tile_pool(name="attp", bufs=2) as attp,
            tc.tile_pool(name="treep", bufs=2) as treep,
            tc.tile_pool(name="ropep", bufs=2) as ropep,
            tc.tile_pool(name="rcpp", bufs=2) as rcpp,
            tc.tile_pool(name="atp", bufs=2) as atp,
            tc.tile_pool(name="gtp", bufs=2) as gtp,
            tc.tile_pool(name="otp", bufs=2) as otp,
            tc.tile_pool(name="psP", bufs=5, space="PSUM") as psP,
            tc.tile_pool(name="poP", bufs=2, space="PSUM") as poP,
            tc.tile_pool(name="psV", bufs=1, space="PSUM") as psV,
        ):
            ones128 = glob.tile([128, 128], bf16, name="ones128")
            nc.vector.memset(ones128[:], 1.0)

            wq_sb = glob.tile([128, DCH, OSL], bf16, name="wq_sb")
            wk_sb = glob.tile([128, DCH, OSL], bf16, name="wk_sb")
            wv_sb = glob.tile([128, DCH, OSL], bf16, name="wv_sb")
            wo_sb = glob.tile([128, DCH, OSL], bf16, name="wo_sb")
            cosb = glob.tile([128, S], bf16, name="cosb")
            sinb = glob.tile([128, S], bf16, name="sinb")
            # startup order: what the first projection needs, first
            nc.gpsimd.dma_start(wq_sb[:], wqS[:, :, :])
            xt00 = xtp.tile([128, DCH, SCH], bf16, name="xt00", tag="xt")
            nc.gpsimd.dma_start(xt00[:], xS[:, 0, :, :])
            nc.gpsimd.dma_start(cosb[:], cosd[:, :])
            nc.gpsimd.dma_start(sinb[:], sind[:, :])
            nc.gpsimd.dma_start(wk_sb[:], wkS[:, :, :])
            nc.gpsimd.dma_start(wv_sb[:], wvS[:, :, :])
            nc.gpsimd.dma_start(wo_sb[:], woS[:, :, :])

            bounce = [dram.tile([OSL, HB], bf16, name=f"bounce{u}")
                      for u in range(NU)]
            gath = [dram.tile([NCORES * OSL, HB], bf16, addr_space="Shared",
                              name=f"gath{u}") for u in range(NU)]

            def fetch_x(b, sc):
                xt = xtp.tile([128, DCH, SCH], bf16, name="xt", tag="xt")
                nc.gpsimd.dma_start(xt[:], xS[:, b * (S // SCH) + sc, :, :])
                return xt

            proj_result = {}

            def proj_chunks(b, xt_first):
                """Yield 24 chunk-closures projecting batch b (q/k/v)."""
                qt = qkv.tile([128, HPC, S], bf16, name="qt", tag="qt")
                kt = qkv.tile([128, HPC, S], bf16, name="kt", tag="kt")
                vt = qkv.tile([128, KB, HPC, HD], bf16, name="vt", tag="vt")
                proj_result[b] = (qt, kt, vt)
                state = {"xt": xt_first, "xt_next": None}

                def qk_chunk(sc, w_sb, dstT, h):
                    xt = state["xt"]
                    sl = slice(sc * SCH, (sc + 1) * SCH)
                    cosr = cosb[:, sl]
                    sinr = sinb[:, sl]
                    ps = psP.tile([128, SCH], fp32, name="ps", tag="ps")
                    for c in range(DCH):
                        nc.tensor.matmul(
                            ps[:], w_sb[:, c, h * HD:(h + 1) * HD],
                            xt[:, c, :], start=(c == 0), stop=(c == DCH - 1))
                    m1 = ropep.tile([128, SCH], bf16, name="m1", tag="m1")
                    m2 = ropep.tile([128, SCH], bf16, name="m2", tag="m2")
                    # m1 = [tr*cos ; ti*cos]; m2 swapped-halves =
                    # [ti*sin ; tr*sin] so the DVE add/sub below uses equal
                    # SBUF base partitions (the PSUM operand may differ).
                    nc.vector.tensor_mul(m1[:], ps[:], cosr)
                    nc.vector.tensor_mul(
                        m2[0:64, :], ps[64:128, :], sinr[0:64, :])
                    nc.vector.tensor_mul(
                        m2[64:128, :], ps[0:64, :], sinr[64:128, :])
                    nc.vector.tensor_sub(
                        dstT[0:64, h, sl], m1[0:64, :], m2[0:64, :])
                    nc.vector.tensor_add(
                        dstT[64:128, h, sl], m2[64:128, :], m1[64:128, :])

                def v_chunk(sc, pair):
                    xt = state["xt"]
                    psv = psV.tile([128, 2, OSL], fp32, name="psv")
                    for j in range(2):
                        ssb = pair * 2 + j
                        for c in range(DCH):
                            nc.tensor.matmul(
                                psv[:, j, :],
                                xt[:, c, ssb * 128:(ssb + 1) * 128],
                                wv_sb[:, c, :],
                                start=(c == 0), stop=(c == DCH - 1))
                    kb0 = sc * 4 + pair * 2
                    nc.vector.tensor_copy(
                        vt[:, kb0:kb0 + 2, :, :],
                        psv[:].rearrange("p s (h d) -> p s h d", h=HPC))

                def make(fn, *args):
                    return lambda: fn(*args)

                def first_chunk(sc):
                    # rotate in the prefetched x tile and prefetch the next
                    if sc > 0:
                        state["xt"] = state["xt_next"]
                    if sc + 1 < S // SCH:
                        state["xt_next"] = fetch_x(b, sc + 1)
                    qk_chunk(sc, wq_sb, qt, 0)

                for sc in range(S // SCH):
                    yield make(first_chunk, sc)
                    yield make(qk_chunk, sc, wq_sb, qt, 1)
                    yield make(qk_chunk, sc, wk_sb, kt, 0)
                    yield make(qk_chunk, sc, wk_sb, kt, 1)
                    yield make(v_chunk, sc, 0)
                    yield make(v_chunk, sc, 1)

            def allgather(u):
                nc.gpsimd.collective_compute(
                    "AllGather",
                    mybir.AluOpType.bypass,
                    ins=[bounce[u].opt()],
                    outs=[gath[u].opt()],
                    replica_groups=[list(range(NCORES))],
                )

            def wo_half(b, half):
                u = b * 2 + half
                for rc_ in range(2):
                    gh = []
                    for dh in range(2):
                        g = gtp.tile([128, DCH // 2, QC], bf16, name="gt",
                                     tag="gt")
                        nc.sync.dma_start(
                            g[:],
                            gath[u][dh * 1024:(dh + 1) * 1024,
                                    rc_ * QC:(rc_ + 1) * QC]
                            .rearrange("(c p) n -> p c n", p=128))
                        gh.append(g)
                    for oc in range(OSL // 128):
                        psw = psP.tile([128, QC], fp32, name="psw", tag="ps")
                        for c in range(DCH):
                            nc.tensor.matmul(
                                psw[:],
                                wo_sb[:, c, oc * 128:(oc + 1) * 128],
                                gh[c // 8][:, c % 8, :],
                                start=(c == 0), stop=(c == DCH - 1))
                        out_t = otp.tile([128, QC], fp32, name="out_t")
                        nc.vector.tensor_copy(out_t[:], psw[:])
                        col0 = b * S + half * HB + rc_ * QC
                        nc.sync.dma_start(
                            outp[oc * 128:(oc + 1) * 128, col0:col0 + QC],
                            out_t[:])

            # --- attention with software pipelining -------------------
            # flush state: pending normalize of the previous unit
            pend = []

            def flush_one():
                if not pend:
                    return
                b, h, qc, po, dsum = pend.pop(0)
                psd = psP.tile([128, QC], fp32, name="psd", tag="ps")
                nc.tensor.matmul(psd[:], ones128[:], dsum[:],
                                 start=True, stop=True)
                rcp = rcpp.tile([128, QC], fp32, name="rcp")
                nc.vector.reciprocal(rcp[:], psd[:])
                a_t = atp.tile([128, QC], bf16, name="a_t")
                nc.vector.tensor_mul(a_t[:], po[:], rcp[:])
                u = b * 2 + qc // 2
                col0 = (qc % 2) * QC
                nc.gpsimd.dma_start(
                    bounce[u][h * HD:(h + 1) * HD, col0:col0 + QC], a_t[:])
                if h == 1 and qc % 2 == 1:
                    allgather(u)

            def attn_unit(b, qt, kt, vt, h, qc, fillers):
                expT = attp.tile([128, KB, QC], bf16, name="expT")
                for kb in range(KB):
                    pss = psP.tile([128, QC], fp32, name="pss", tag="ps")
                    nc.tensor.matmul(
                        pss[:],
                        kt[:, h, kb * 128:(kb + 1) * 128],
                        qt[:, h, qc * QC:(qc + 1) * QC],
                        start=True, stop=True)
                    nc.scalar.activation(
                        expT[:, kb, :], pss[:],
                        mybir.ActivationFunctionType.Exp,
                        scale=inv_sqrt_hd)
                # normalize/ship the previous unit while exps stream
                flush_one()
                # interleave independent matmul work so the attn@v group
                # below never waits on the ScalarE exp chain
                for f in fillers:
                    f()
                po = poP.tile([128, QC], fp32, name="po", tag="po")
                for kb in range(KB):
                    nc.tensor.matmul(
                        po[:], vt[:, kb, h, 0:HD], expT[:, kb, :],
                        start=(kb == 0), stop=(kb == KB - 1))
                # denominator: tree-sum the 16 k-blocks (gpsimd+vector)
                s1 = treep.tile([128, 4, QC], bf16, name="s1", tag="s1")
                s2 = treep.tile([128, 4, QC], bf16, name="s2", tag="s2")
                s4 = treep.tile([128, 2, QC], bf16, name="s4", tag="s4")
                dsum = treep.tile([128, QC], bf16, name="dsum", tag="ds")
                nc.gpsimd.tensor_add(s1[:], expT[:, 0:4, :], expT[:, 4:8, :])
                nc.vector.tensor_add(s2[:], expT[:, 8:12, :],
                                     expT[:, 12:16, :])
                nc.vector.tensor_add(s1[:], s1[:], s2[:])
                nc.gpsimd.tensor_add(s4[:], s1[:, 0:2, :], s1[:, 2:4, :])
                nc.vector.tensor_add(dsum[:], s4[:, 0, :], s4[:, 1, :])
                pend.append((b, h, qc, po, dsum))

            # ---------------- main schedule ---------------------------
            # proj(0) runs unfilled upfront; proj(b+1) and wo(b-1)
            # quarters fill the attention(b) units.
            gens = {}
            gens[0] = proj_chunks(0, xt00)
            for f in gens[0]:
                f()  # issue all 24 chunks of batch 0 now

            for b in range(B):
                if b + 1 < B:
                    xt_n = fetch_x(b + 1, 0)
                    gens[b + 1] = proj_chunks(b + 1, xt_n)
                qt, kt, vt = proj_result[b]
                # wo halves available to fill batch b's units
                # (half1's AllGather of batch b-1 is issued during unit 0)
                wo_sched = {}
                if b >= 1:
                    wo_sched[1] = [(b - 1, 0)]
                    wo_sched[4] = [(b - 1, 1)]
                if b == B - 1:
                    # this batch's half0 AllGather is issued during unit 4
                    wo_sched[6] = [(b, 0)]
                units = [(h, qc) for qc in range(NQC) for h in range(HPC)]
                for ui, (h, qc) in enumerate(units):
                    fillers = []
                    if b + 1 < B:
                        g = gens[b + 1]
                        for _ in range(3):
                            try:
                                fillers.append(next(g))
                            except StopIteration:
                                break
                    for (wb, wq) in wo_sched.get(ui, []):
                        fillers.append(
                            lambda wb=wb, wq=wq: wo_half(wb, wq))
                    attn_unit(b, qt, kt, vt, h, qc, fillers)
                if b + 1 < B:
                    # drain any leftover proj chunks of b+1
                    for f in gens[b + 1]:
                        f()
            flush_one()
            wo_half(B - 1, 1)

    nc.compile()
    return nc


def _shard_inputs(x, freqs_cos, freqs_sin, wq, wk, wv, wo):
    xf = np.asarray(x, dtype=np.float32).reshape(ROWS, D)
    xT = np.ascontiguousarray(xf.T).astype(BF16)  # [D, ROWS]
    # pre-shuffle: xS[p, blk, c, j] = xT[c*128+p, blk*512+j]
    xS = np.ascontiguousarray(
        xT.reshape(DCH, 128, NBLK, SCH).transpose(1, 2, 0, 3))
    fcT = np.asarray(freqs_cos, dtype=np.float32).T  # [64, S]
    fsT = np.asarray(freqs_sin, dtype=np.float32).T
    cosd = np.ascontiguousarray(np.concatenate([fcT, fcT], 0)).astype(BF16)
    sind = np.ascontiguousarray(np.concatenate([fsT, fsT], 0)).astype(BF16)
    # even indices (real half) then odd (imag half), per head
    perm = np.concatenate([np.arange(0, HD, 2), np.arange(1, HD, 2)])

    def shuf(wrows):  # [OSL, D] -> [128, DCH, OSL]
        wT = np.ascontiguousarray(np.asarray(wrows, dtype=np.float32).T)
        return np.ascontiguousarray(
            wT.reshape(DCH, 128, OSL).transpose(1, 0, 2)).astype(BF16)

    in_maps = []
    for c in range(NCORES):
        rows = slice(c * OSL, (c + 1) * OSL)
        wq_c = np.asarray(wq)[rows].reshape(HPC, HD, D)[:, perm, :]
        wk_c = np.asarray(wk)[rows].reshape(HPC, HD, D)[:, perm, :]
        in_maps.append({
            "xS": xS,
            "wqS": shuf(wq_c.reshape(OSL, D)),
            "wkS": shuf(wk_c.reshape(OSL, D)),
            "wvS": shuf(np.asarray(wv)[rows]),
            "woS": shuf(np.asarray(wo)[rows]),
            "cosd": cosd,
            "sind": sind,
        })
    return in_maps


def run(inputs, trace=False, trace_cores=None):
    """Build (cached), run on 8 cores; returns (full_output, results)."""
    global _NC_CACHE
    from concourse.bass_utils import run_bass_kernel_spmd
    if _NC_CACHE is None:
        _NC_CACHE = _build()
    in_maps = _shard_inputs(**inputs)
    res = run_bass_kernel_spmd(
        _NC_CACHE, in_maps, core_ids=list(range(NCORES)), trace=trace,
        trace_cores=trace_cores)
    parts = [np.ascontiguousarray(
        np.asarray(res.results[c]["out"], dtype=np.float32).T)
        for c in range(NCORES)]
    full = np.concatenate(parts, axis=1).reshape(B, S, D)
    return full, res


def kernel(x, freqs_cos, freqs_sin, wq, wk, wv, wo):
    full, _ = run(dict(x=x, freqs_cos=freqs_cos, freqs_sin=freqs_sin,
                       wq=wq, wk=wk, wv=wv, wo=wo))
    return full
